# revision 70
# baseline (speedup 1.0000x reference)
"""MoE (top-2 of 8 experts, SwiGLU FFN) on 8 Trainium2 NeuronCores.

Strategy (expert-parallel + accuracy-guided precision split):
 - Host: router matmul (f64) + top-2 + softmax gates.  One expert per
   core.  Each expert keeps exactly C16 tokens on the fp16 path; the
   excess tokens (those with the SMALLEST gate weights, which bound
   their error contribution) spill to an fp8 (e4m3, DoubleRow 2x)
   path of capacity C8.  This simultaneously load-balances the cores
   (no fp16 padding at all) and keeps the global rel-err ~1.2e-2
   (tolerance 2e-2): error from an fp8 pair scales with its gate g,
   and the spill picks per-expert the smallest-g secondary pairs.
 - Device (per core), feature-major layout so per-feature biases are
   per-partition scalars:
     fp16 pass over C16 tokens:  hT = W1[e] @ xgT  (PE fp16->f32 PSUM)
         aT = h1 * silu(h2)   (ACT + DVE)
         yT = W2[e] @ aT      (PE, ACT -> f16 out)
     fp8 pass over C8 tokens: same, matmuls in e4m3 DoubleRow mode
         (2 contraction subtiles per pass = 2x rate), scales folded
         into the host-side quantization (x*16, W*512, a*8).
 - Host: gather back, gate-weight, scatter-add (f32).

DMA: host pre-packs every stream so each descriptor moves 2-8KB
contiguous runs per partition.  Exactly TWO HW queues (sync, gpsimd)
stream at any time — a third concurrent queue caps the PE clock at
~2GHz — with late-needed streams (w2, fp8 weights) sequenced behind
the critical ones via each queue's FIFO.  The scalar engine stays
DMA-free: queued DMA issues hit semaphore-reuse waits that would
block its activations (and with them PSUM recycling and the PE).

Shapes hardcoded for the problem: x [2,2048,1024], E=8, K=2, D=1024,
F=2048.
"""

import os

import numpy as np
import ml_dtypes

import concourse.bacc as bacc
import concourse.tile as tile
from concourse import mybir
from concourse.bass_utils import run_bass_kernel_spmd

B, S, D = 2, 2048, 1024
T = B * S
E = 8
K = 2
F = 2048
TWOF = 2 * F
KT_D = D // 128    # 8 contraction subtiles for matmul 1
KT_F = F // 128    # 16 contraction subtiles for matmul 2
NF1 = TWOF // 128  # 32 output feature chunks of matmul 1 (16 pairs)
NF2 = D // 128     # 8 output feature chunks of matmul 2
NT = 512           # fp16 token tile (matmul moving free dim)
SPILL = 288        # max fp8 spill per expert (sets C16); the host
                   # backs this off if the predicted spill error for
                   # the actual gate distribution exceeds ERR_CAP
EPS8 = 0.059       # measured rel-err of a fully-fp8 token pair
ERR_CAP = 0.0188   # error budget (tolerance 2e-2 with margin)

XS = 16.0          # x fp8 scale
WS = 512.0         # weight fp8 scale
AS = 8.0           # activation fp8 scale;  a8 = ps1*(AS/(XS*WS)) * silu
OS = AS * WS       # fp8 output scale: o8 = OS * y

_NC_CACHE = {}
_W_CACHE = {}


def _tiles16(C):
    """fp16 token tiles: a small head tile (its xg lands early so the
    PE starts sooner), remainder in near-equal <=512 chunks (multiples
    of 16).  The head tile must stay >=~350 so mm1's w1-chunk
    consumption period does not outrun the two-queue DMA delivery."""
    first = min(352, C)
    szs = [first]
    rem = C - first
    if rem > 0:
        n = -(-rem // NT)
        base = -(-(rem // n) // 16) * 16
        while rem > 0:
            sz = min(base, rem)
            if 0 < rem - sz < 64:
                sz = rem
            szs.append(sz)
            rem -= sz
    tiles = []
    off = 0
    for sz in szs:
        tiles.append((off, sz))
        off += sz
    return tiles


def _build(C16, C8, zero_b1=True, zero_b2=True):
    nc = bacc.Bacc(None, target_bir_lowering=False)
    f16, f32, f8 = mybir.dt.float16, mybir.dt.float32, mybir.dt.float8e4
    DR = mybir.MatmulPerfMode.DoubleRow
    Mult = mybir.AluOpType.mult

    tiles = _tiles16(C16)
    ntile = len(tiles)
    NTA = max(sz for _, sz in tiles)  # widest fp16 tile (SBUF sizing)
    NTW = max(NTA, C8)                # work-tile width for s/o pools

    # ---- DRAM tensors (host-packed for contiguous DMA runs)
    xg_d = [nc.dram_tensor(f"xg{t}", [128, KT_D, sz], f16, kind="ExternalInput")
            for t, (_, sz) in enumerate(tiles)]
    w1p = nc.dram_tensor("w1p", [KT_F, 128, KT_D, 256], f16, kind="ExternalInput")
    w2p = nc.dram_tensor("w2p", [4, 128, KT_F, 256], f16, kind="ExternalInput")
    yt_d = [nc.dram_tensor(f"yt{t}", [NF2, 128, sz], f16, kind="ExternalOutput")
            for t, (_, sz) in enumerate(tiles)]
    if C8:
        xq8 = nc.dram_tensor("xq8", [128, KT_D // 2, 2, C8], f8, kind="ExternalInput")
        w1q = nc.dram_tensor("w1q", [KT_F, 128, KT_D // 2, 2, 256], f8,
                             kind="ExternalInput")
        w2q = nc.dram_tensor("w2q", [4, 128, KT_F // 2, 2, 256], f8,
                             kind="ExternalInput")
        yq8_d = nc.dram_tensor("yq8", [NF2, 128, C8], f16, kind="ExternalOutput")
    if not zero_b1:
        b1c = nc.dram_tensor("b1c", [128, NF1], f32, kind="ExternalInput")
    if not zero_b2:
        b2c = nc.dram_tensor("b2c", [128, NF2], f32, kind="ExternalInput")
        if C8:
            b2q = nc.dram_tensor("b2q", [128, NF2], f32, kind="ExternalInput")

    Silu = mybir.ActivationFunctionType.Silu
    Ident = mybir.ActivationFunctionType.Identity

    with tile.TileContext(nc) as tc:
        with (
            tc.tile_pool(name="wpool", bufs=1) as wpool,
            tc.tile_pool(name="apool", bufs=2) as apool,
            tc.tile_pool(name="tpool", bufs=3) as tpool,
            tc.tile_pool(name="opool", bufs=4) as opool,
            tc.tile_pool(name="psA", bufs=3, space="PSUM") as psA,
            tc.tile_pool(name="psB", bufs=2, space="PSUM") as psB,
        ):
            # Resident weights / activations
            w1_sb = wpool.tile([128, KT_F, KT_D, 256], f16)
            w2_sb = wpool.tile([128, 4, KT_F, 256], f16)
            xg_sb = [wpool.tile([128, KT_D, sz], f16, name=f"xg_sb{t}")
                     for t, (_, sz) in enumerate(tiles)]
            if C8:
                w1q_sb = wpool.tile([128, KT_F, KT_D // 2, 2, 256], f8)
                w2q_sb = wpool.tile([128, 4, KT_F // 2, 2, 256], f8)
                xq8_sb = wpool.tile([128, KT_D // 2, 2, C8], f8)
                a8_sb = wpool.tile([128, KT_F // 2, 2, C8], f8)
            if not zero_b1:
                b1_sb = wpool.tile([128, NF1], f32)
            if not zero_b2:
                b2_sb = wpool.tile([128, NF2], f32)
                if C8:
                    b2q_sb = wpool.tile([128, NF2], f32)

            # Warm-up matmuls on a zeroed tile keep the PE busy (and its
            # clock ramping to 2.4GHz) during the initial DMA wait; an
            # idle PE hole >~5us here can throttle the clock to ~2GHz
            # for the whole kernel, so the count is sized to end just as
            # the first real inputs land.
            warm_sb = wpool.tile([128, 128], f16)
            nc.vector.memset(warm_sb, 0.0)
            # Dummy Silu forces the lazy ACT_TABLE_LOAD (1.3us) to happen
            # now, while scalar is idle, not at tile 0's first real silu.
            warm_s = wpool.tile([128, 1], f32)
            nc.scalar.activation(warm_s, warm_sb[:, 0:1],
                                 mybir.ActivationFunctionType.Silu)
            warm_ps = psB.tile([128, NT], f32, tag="psb")
            for _ in range(95):
                nc.tensor.matmul(warm_ps[:, :128], warm_sb, warm_sb,
                                 start=True, stop=True)

            # ---- DMA issue plan.  NOTE: TileContext schedules by data
            # deps, not python emission order — ordering within a queue
            # comes only from its FIFO, so late-needed streams (w2, fp8
            # weights) are placed BEHIND the critical stream on the same
            # queue.  Strictly two active queues at all times (a third
            # streaming queue caps the PE clock at ~2GHz).  w1 chunks
            # alternate between both queues (mm1-tile0 consumes a chunk
            # every ~2.6us; one queue delivers only ~3.5us/chunk); the
            # two k-halves of xg tile 0 head each queue.
            w1r = w1p.rearrange("i p k c -> p i k c")
            w2r = w2p.rearrange("g p k c -> p g k c")
            nc.gpsimd.dma_start(out=xg_sb[0], in_=xg_d[0][:, :, :])
            nc.sync.dma_start(out=w1_sb[:, 0, :, :], in_=w1r[:, 0, :, :])
            # xg tile 1 slots in after c9: early enough for tile 1's
            # mm1 start, late enough that every w1-chunk deadline holds.
            for i in range(1, 11, 2):
                nc.sync.dma_start(out=w1_sb[:, i, :, :], in_=w1r[:, i, :, :])
            for t in range(1, ntile):
                nc.sync.dma_start(out=xg_sb[t], in_=xg_d[t][:, :, :])
            for i in range(11, KT_F, 2):
                nc.sync.dma_start(out=w1_sb[:, i, :, :], in_=w1r[:, i, :, :])
            for i in range(2, KT_F, 2):
                nc.gpsimd.dma_start(out=w1_sb[:, i, :, :], in_=w1r[:, i, :, :])
            if not zero_b1:
                nc.sync.dma_start(out=b1_sb, in_=b1c[:, :])
            if not zero_b2:
                nc.sync.dma_start(out=b2_sb, in_=b2c[:, :])
                if C8:
                    nc.sync.dma_start(out=b2q_sb, in_=b2q[:, :])
            # w2 split across both queue tails; xq8 and the fp8 weights
            # ride the very back (needed only for the final fp8 pass) so
            # they never delay w2, which mm2-tile0 is waiting on.
            nc.gpsimd.dma_start(out=w2_sb[:, 0, :, :], in_=w2r[:, 0, :, :])
            nc.sync.dma_start(out=w2_sb[:, 2, :, :], in_=w2r[:, 2, :, :])
            nc.gpsimd.dma_start(out=w2_sb[:, 1, :, :], in_=w2r[:, 1, :, :])
            nc.sync.dma_start(out=w2_sb[:, 3, :, :], in_=w2r[:, 3, :, :])
            if C8:
                nc.sync.dma_start(out=xq8_sb, in_=xq8[:, :, :, :])
            if C8:
                w1qr = w1q.rearrange("i p k two c -> p i k two c")
                nc.gpsimd.dma_start(out=w1q_sb[:, 0:8], in_=w1qr[:, 0:8])
                nc.gpsimd.dma_start(out=w1q_sb[:, 8:16], in_=w1qr[:, 8:16])
                w2qr = w2q.rearrange("g p k two c -> p g k two c")
                nc.gpsimd.dma_start(out=w2q_sb, in_=w2qr[:, :, :, :, :])

            # ---- fp16 pass
            for t, (n0, nsz) in enumerate(tiles):
                a_t = apool.tile([128, KT_F, NTA], f16, tag="a")
                for i in range(KT_F):
                    ps1 = psA.tile([128, NT], f32, tag="ps1")
                    ps2 = psA.tile([128, NT], f32, tag="ps2")
                    for k in range(KT_D):
                        nc.tensor.matmul(
                            ps1[:, :nsz], w1_sb[:, i, k, 0:128],
                            xg_sb[t][:, k, :nsz],
                            start=(k == 0), stop=(k == KT_D - 1),
                        )
                    for k in range(KT_D):
                        nc.tensor.matmul(
                            ps2[:, :nsz], w1_sb[:, i, k, 128:256],
                            xg_sb[t][:, k, :nsz],
                            start=(k == 0), stop=(k == KT_D - 1),
                        )
                    s_t = tpool.tile([128, NTW], f32, tag="s")
                    if zero_b1:
                        nc.scalar.activation(s_t[:, :nsz], ps2[:, :nsz], Silu)
                        nc.vector.tensor_mul(
                            a_t[:, i, :nsz], ps1[:, :nsz], s_t[:, :nsz])
                    else:
                        nc.scalar.activation(
                            s_t[:, :nsz], ps2[:, :nsz], Silu,
                            bias=b1_sb[:, KT_F + i:KT_F + i + 1])
                        nc.vector.scalar_tensor_tensor(
                            a_t[:, i, :nsz], ps1[:, :nsz],
                            b1_sb[:, i:i + 1], s_t[:, :nsz],
                            mybir.AluOpType.add, Mult)
                for j in range(NF2):
                    ps = psB.tile([128, NT], f32, tag="psb")
                    for kf in range(KT_F):
                        nc.tensor.matmul(
                            ps[:, :nsz],
                            w2_sb[:, j // 2, kf, (j % 2) * 128:(j % 2) * 128 + 128],
                            a_t[:, kf, :nsz],
                            start=(kf == 0), stop=(kf == KT_F - 1),
                        )
                    o_t = opool.tile([128, NTW], f16, tag="o")
                    if zero_b2:
                        nc.scalar.activation(o_t[:, :nsz], ps[:, :nsz], Ident)
                    else:
                        nc.scalar.activation(o_t[:, :nsz], ps[:, :nsz], Ident,
                                             bias=b2_sb[:, j:j + 1])
                    # all fp16 outs ride sync: it is idle once the loads
                    # finish, while gpsimd still streams the fp8 weights
                    # — an out-DMA queued behind that bulk would delay
                    # o_t recycling and stall mm2 via the ident chain.
                    nc.sync.dma_start(out=yt_d[t][j, :, :], in_=o_t[:, :nsz])

            # ---- fp8 spill pass (DoubleRow e4m3, 2x rate)
            if C8:
                for i in range(KT_F):
                    ps1 = psA.tile([128, NT], f32, tag="ps1")
                    ps2 = psA.tile([128, NT], f32, tag="ps2")
                    for k2 in range(KT_D // 2):
                        nc.tensor.matmul(
                            ps1[:, :C8], w1q_sb[:, i, k2, :, 0:128],
                            xq8_sb[:, k2, :, :],
                            start=(k2 == 0), stop=(k2 == KT_D // 2 - 1),
                            perf_mode=DR,
                        )
                    for k2 in range(KT_D // 2):
                        nc.tensor.matmul(
                            ps2[:, :C8], w1q_sb[:, i, k2, :, 128:256],
                            xq8_sb[:, k2, :, :],
                            start=(k2 == 0), stop=(k2 == KT_D // 2 - 1),
                            perf_mode=DR,
                        )
                    s_t = tpool.tile([128, NTW], f32, tag="s")
                    if zero_b1:
                        nc.scalar.activation(s_t[:, :C8], ps2[:, :C8], Silu,
                                             scale=float(1.0 / (XS * WS)))
                    else:
                        nc.scalar.activation(s_t[:, :C8], ps2[:, :C8], Silu,
                                             scale=float(1.0 / (XS * WS)),
                                             bias=b1_sb[:, KT_F + i:KT_F + i + 1])
                        # NOTE: nonzero b1 with fp8 needs b1 un-scaled here;
                        # bias applies after scale, so this is h2 + b1b. OK.
                    # a8 = (ps1 * AS/(XS*WS)) * silu  -> e4m3
                    if zero_b1:
                        nc.vector.scalar_tensor_tensor(
                            a8_sb[:, i // 2, i % 2, :], ps1[:, :C8],
                            float(AS / (XS * WS)), s_t[:, :C8], Mult, Mult)
                    else:
                        h_t = tpool.tile([128, NTW], f32, tag="h8")
                        nc.scalar.activation(h_t[:, :C8], ps1[:, :C8], Ident,
                                             scale=float(1.0 / (XS * WS)),
                                             bias=b1_sb[:, i:i + 1])
                        nc.vector.scalar_tensor_tensor(
                            a8_sb[:, i // 2, i % 2, :], h_t[:, :C8],
                            float(AS), s_t[:, :C8], Mult, Mult)
                o8_sb = wpool.tile([128, NF2, C8], f16)
                yq8r = yq8_d.rearrange("j p c -> p j c")
                for j in range(NF2):
                    ps = psB.tile([128, NT], f32, tag="psb")
                    for kf2 in range(KT_F // 2):
                        nc.tensor.matmul(
                            ps[:, :C8],
                            w2q_sb[:, j // 2, kf2, :,
                                   (j % 2) * 128:(j % 2) * 128 + 128],
                            a8_sb[:, kf2, :, :],
                            start=(kf2 == 0), stop=(kf2 == KT_F // 2 - 1),
                            perf_mode=DR,
                        )
                    if zero_b2:
                        nc.scalar.activation(o8_sb[:, j, :], ps[:, :C8], Ident)
                    else:
                        # bias is b2*OS (host pre-scales b2q by OS)
                        nc.scalar.activation(o8_sb[:, j, :], ps[:, :C8], Ident,
                                             bias=b2q_sb[:, j:j + 1])
                    # two batched out DMAs compress the end-of-kernel
                    # drain (each issue + queue-drain costs ~0.6us)
                    # both batches on sync: its queue stays warm from the
                    # fp16 outs, so the final descriptors prefetch instead
                    # of paying the ~2.8us cold-fetch latency; gpsimd's
                    # end-of-kernel drain is then empty.
                    if j == 5:
                        nc.sync.dma_start(out=yq8r[:, 0:6, :],
                                          in_=o8_sb[:, 0:6, :])
                    elif j == 7:
                        nc.sync.dma_start(out=yq8r[:, 6:8, :],
                                          in_=o8_sb[:, 6:8, :])
    nc.compile()
    return nc


def _get_nc(C16, C8, zero_b1, zero_b2):
    key = (C16, C8, zero_b1, zero_b2)
    nc = _NC_CACHE.get(key)
    if nc is None:
        nc = _build(C16, C8, zero_b1, zero_b2)
        _NC_CACHE[key] = nc
    return nc


def _q8(v, s):
    return np.clip(v * s, -240.0, 240.0).astype(ml_dtypes.float8_e4m3)


def _pack_weights(W1, W2):
    key = (W1.shape, W2.shape, W1.dtype.str,
           bytes(np.asarray(W1[0, 0, :8]).data),
           bytes(np.asarray(W2[0, 0, :8]).data))
    hit = _W_CACHE.get("w")
    if hit is not None and hit[0] == key:
        return hit[1]
    packs = []
    for e in range(E):
        W1T = np.ascontiguousarray(W1[e].T)            # [D, 2F] f32
        W2T = np.ascontiguousarray(W2[e].T)            # [F, D]  f32
        W1T16 = W1T.astype(np.float16)
        W2T16 = W2T.astype(np.float16)
        # w1p[i, p, k, c]: chunk i = (h1[i*128:...], h2[i*128:...])
        w1p = np.ascontiguousarray(
            W1T16.reshape(KT_D, 128, 2, KT_F, 128)
            .transpose(3, 1, 0, 2, 4).reshape(KT_F, 128, KT_D, 256))
        # w2p[g, p, k, c]: d-block g (covers output chunks 2g, 2g+1)
        w2p = np.ascontiguousarray(
            W2T16.reshape(KT_F, 128, 4, 256).transpose(2, 1, 0, 3))
        W1q = _q8(W1T, WS)
        W2q = _q8(W2T, WS)
        w1q = np.ascontiguousarray(
            W1q.reshape(KT_D // 2, 2, 128, 2, KT_F, 128)
            .transpose(4, 2, 0, 1, 3, 5).reshape(KT_F, 128, KT_D // 2, 2, 256))
        w2q = np.ascontiguousarray(
            W2q.reshape(KT_F // 2, 2, 128, 4, 256).transpose(3, 2, 0, 1, 4))
        packs.append((w1p, w2p, w1q, w2q))
    _W_CACHE["w"] = (key, packs)
    return packs


def kernel(x, Wr, temp, W1, b1, W2, b2):
    x = np.asarray(x)
    xf = np.ascontiguousarray(x.reshape(T, D), dtype=np.float32)

    # ---- host router (f64 for a stable top-k ordering)
    logits = xf.astype(np.float64) @ np.asarray(Wr).astype(np.float64).T
    logits /= np.float64(np.asarray(temp).reshape(-1)[0])
    top_idx = np.argsort(-logits, axis=1, kind="stable")[:, :K]  # [T, K]
    top_v = np.take_along_axis(logits, top_idx, axis=1)
    top_v -= top_v.max(axis=1, keepdims=True)
    exv = np.exp(top_v)
    gates = (exv / exv.sum(axis=1, keepdims=True)).astype(np.float64)  # [T, K]

    # ---- dispatch: per-expert fp16 lists (exactly C16 tokens) and fp8
    # spill lists (smallest-gate secondary pairs)
    prim_rows = [np.where(top_idx[:, 0] == e)[0] for e in range(E)]
    sec_rows = [np.where(top_idx[:, 1] == e)[0] for e in range(E)]
    counts = np.array([len(prim_rows[e]) + len(sec_rows[e]) for e in range(E)])
    maxprim = max(len(p) for p in prim_rows)
    S_tot = float((gates ** 2).sum())
    spill = SPILL
    while True:
        C16 = int(-(-max(16, counts.max() - spill) // 16) * 16)
        C16 = max(C16, int(-(-maxprim // 16) * 16), 256)
        n8s = [max(0, int(c) - C16) for c in counts]
        C8 = int(-(-max(n8s) // 16) * 16) if max(n8s) > 0 else 0
        if spill <= 0 or C8 == 0:
            break
        # predicted global rel-err from spilling the smallest-gate
        # secondary pairs (error of a pair scales with its gate)
        S8 = 0.0
        for e in range(E):
            gs = np.sort(gates[sec_rows[e], 1])[:n8s[e]]
            S8 += float((gs ** 2).sum())
        if EPS8 * np.sqrt(S8 / S_tot) <= ERR_CAP:
            break
        spill -= 32

    idx16, gate16, idx8, gate8 = [], [], [], []
    for e in range(E):
        gp = gates[prim_rows[e], 0]
        gs = gates[sec_rows[e], 1]
        order = np.argsort(gs, kind="stable")
        n8 = n8s[e]
        idx8.append(sec_rows[e][order[:n8]])
        gate8.append(gs[order[:n8]])
        idx16.append(np.concatenate([prim_rows[e], sec_rows[e][order[n8:]]]))
        gate16.append(np.concatenate([gp, gs[order[n8:]]]))

    b1a = np.asarray(b1, dtype=np.float32)
    b2a = np.asarray(b2, dtype=np.float32)
    zero_b1 = not b1a.any()
    zero_b2 = not b2a.any()
    nc = _get_nc(C16, C8, zero_b1, zero_b2)
    tiles = _tiles16(C16)

    xf16 = xf.astype(np.float16)
    packs = _pack_weights(np.asarray(W1, dtype=np.float32),
                          np.asarray(W2, dtype=np.float32))

    in_maps = []
    for e in range(E):
        w1p, w2p, w1q, w2q = packs[e]
        m = {"w1p": w1p, "w2p": w2p}
        xg = np.zeros((C16, D), np.float16)
        xg[:len(idx16[e])] = xf16[idx16[e]]
        for t, (n0, nsz) in enumerate(tiles):
            m[f"xg{t}"] = np.ascontiguousarray(
                xg[n0:n0 + nsz].reshape(nsz, KT_D, 128).transpose(2, 1, 0))
        if C8:
            x8 = np.zeros((C8, D), ml_dtypes.float8_e4m3)
            x8[:len(idx8[e])] = _q8(xf[idx8[e]], XS)
            m["xq8"] = np.ascontiguousarray(
                x8.reshape(C8, KT_D // 2, 2, 128).transpose(3, 1, 2, 0))
            m["w1q"] = w1q
            m["w2q"] = w2q
        if not zero_b1:
            m["b1c"] = np.ascontiguousarray(b1a[e].reshape(NF1, 128).T)
        if not zero_b2:
            m["b2c"] = np.ascontiguousarray(b2a[e].reshape(NF2, 128).T)
            if C8:
                m["b2q"] = np.ascontiguousarray(
                    b2a[e].reshape(NF2, 128).T * OS)
        in_maps.append(m)

    kwargs = {}
    if os.environ.get("KERNEL_TRACE"):
        kwargs = {"trace": True}
    try:
        res = run_bass_kernel_spmd(nc, in_maps, core_ids=list(range(E)), **kwargs)
    except ModuleNotFoundError:
        os.environ["BASS_NEVER_TRACE"] = "1"
        res = run_bass_kernel_spmd(nc, in_maps, core_ids=list(range(E)))
    global LAST_RESULT
    LAST_RESULT = res

    out = np.zeros((T, D), np.float64)
    for e in range(E):
        r = res.results[e]
        n16 = len(idx16[e])
        y16 = np.concatenate(
            [r[f"yt{t}"].transpose(2, 0, 1).reshape(nsz, D)
             for t, (n0, nsz) in enumerate(tiles)], axis=0)[:n16]
        out[idx16[e]] += gate16[e][:, None] * y16.astype(np.float64)
        n8 = len(idx8[e])
        if n8:
            y8 = r["yq8"].transpose(2, 0, 1).reshape(C8, D)[:n8]
            out[idx8[e]] += gate8[e][:, None] * (y8.astype(np.float64) / OS)
    return out.reshape(B, S, D).astype(np.float32)


LAST_RESULT = None


# revision 71
# speedup vs baseline: 1.0155x; 1.0155x over previous
"""MoE (top-2 of 8 experts, SwiGLU FFN) on 8 Trainium2 NeuronCores.

Strategy (expert-parallel + accuracy-guided precision split):
 - Host: router matmul (f64) + top-2 + softmax gates.  One expert per
   core.  Each expert keeps exactly C16 tokens on the fp16 path; the
   excess tokens (those with the SMALLEST gate weights, which bound
   their error contribution) spill to an fp8 (e4m3, DoubleRow 2x)
   path of capacity C8.  This simultaneously load-balances the cores
   (no fp16 padding at all) and keeps the global rel-err ~1.2e-2
   (tolerance 2e-2): error from an fp8 pair scales with its gate g,
   and the spill picks per-expert the smallest-g secondary pairs.
 - Device (per core), feature-major layout so per-feature biases are
   per-partition scalars:
     fp16 pass over C16 tokens:  hT = W1[e] @ xgT  (PE fp16->f32 PSUM)
         aT = h1 * silu(h2)   (ACT + DVE)
         yT = W2[e] @ aT      (PE, ACT -> f16 out)
     fp8 pass over C8 tokens: same, matmuls in e4m3 DoubleRow mode
         (2 contraction subtiles per pass = 2x rate), scales folded
         into the host-side quantization (x*16, W*512, a*8).
 - Host: gather back, gate-weight, scatter-add (f32).

DMA: host pre-packs every stream so each descriptor moves 2-8KB
contiguous runs per partition.  Exactly TWO HW queues (sync, gpsimd)
stream at any time — a third concurrent queue caps the PE clock at
~2GHz — with late-needed streams (w2, fp8 weights) sequenced behind
the critical ones via each queue's FIFO.  The scalar engine stays
DMA-free: queued DMA issues hit semaphore-reuse waits that would
block its activations (and with them PSUM recycling and the PE).

Shapes hardcoded for the problem: x [2,2048,1024], E=8, K=2, D=1024,
F=2048.
"""

import os

import numpy as np
import ml_dtypes

import concourse.bacc as bacc
import concourse.tile as tile
from concourse import mybir
from concourse.bass_utils import run_bass_kernel_spmd

B, S, D = 2, 2048, 1024
T = B * S
E = 8
K = 2
F = 2048
TWOF = 2 * F
KT_D = D // 128    # 8 contraction subtiles for matmul 1
KT_F = F // 128    # 16 contraction subtiles for matmul 2
NF1 = TWOF // 128  # 32 output feature chunks of matmul 1 (16 pairs)
NF2 = D // 128     # 8 output feature chunks of matmul 2
NT = 512           # fp16 token tile (matmul moving free dim)
SPILL = 288        # max fp8 spill per expert (sets C16); the host
                   # backs this off if the predicted spill error for
                   # the actual gate distribution exceeds ERR_CAP
EPS8 = 0.059       # measured rel-err of a fully-fp8 token pair
ERR_CAP = 0.0188   # error budget (tolerance 2e-2 with margin)

XS = 16.0          # x fp8 scale
WS = 512.0         # weight fp8 scale
AS = 8.0           # activation fp8 scale;  a8 = ps1*(AS/(XS*WS)) * silu
OS = AS * WS       # fp8 output scale: o8 = OS * y

_NC_CACHE = {}
_W_CACHE = {}


def _tiles16(C):
    """fp16 token tiles: a small head tile (its xg lands early so the
    PE starts sooner), remainder in near-equal <=512 chunks (multiples
    of 16).  The head tile must stay >=~350 so mm1's w1-chunk
    consumption period does not outrun the two-queue DMA delivery."""
    first = min(352, C)
    szs = [first]
    rem = C - first
    if rem > 0:
        n = -(-rem // NT)
        base = -(-(rem // n) // 16) * 16
        while rem > 0:
            sz = min(base, rem)
            if 0 < rem - sz < 64:
                sz = rem
            szs.append(sz)
            rem -= sz
    tiles = []
    off = 0
    for sz in szs:
        tiles.append((off, sz))
        off += sz
    return tiles


def _build(C16, C8, zero_b1=True, zero_b2=True):
    nc = bacc.Bacc(None, target_bir_lowering=False)
    f16, f32, f8 = mybir.dt.float16, mybir.dt.float32, mybir.dt.float8e4
    DR = mybir.MatmulPerfMode.DoubleRow
    Mult = mybir.AluOpType.mult

    tiles = _tiles16(C16)
    ntile = len(tiles)
    NTA = max(sz for _, sz in tiles)  # widest fp16 tile (SBUF sizing)
    NTW = max(NTA, C8)                # work-tile width for s/o pools

    # ---- DRAM tensors (host-packed for contiguous DMA runs)
    xg_d = [nc.dram_tensor(f"xg{t}", [128, KT_D, sz], f16, kind="ExternalInput")
            for t, (_, sz) in enumerate(tiles)]
    w1p = nc.dram_tensor("w1p", [KT_F, 128, KT_D, 256], f16, kind="ExternalInput")
    w2p = nc.dram_tensor("w2p", [4, 128, KT_F, 256], f16, kind="ExternalInput")
    yt_d = [nc.dram_tensor(f"yt{t}", [NF2, 128, sz], f16, kind="ExternalOutput")
            for t, (_, sz) in enumerate(tiles)]
    if C8:
        xq8 = nc.dram_tensor("xq8", [128, KT_D // 2, 2, C8], f8, kind="ExternalInput")
        w1q = nc.dram_tensor("w1q", [KT_F, 128, KT_D // 2, 2, 256], f8,
                             kind="ExternalInput")
        w2q = nc.dram_tensor("w2q", [4, 128, KT_F // 2, 2, 256], f8,
                             kind="ExternalInput")
        yq8_d = nc.dram_tensor("yq8", [NF2, 128, C8], f16, kind="ExternalOutput")
    if not zero_b1:
        b1c = nc.dram_tensor("b1c", [128, NF1], f32, kind="ExternalInput")
    if not zero_b2:
        b2c = nc.dram_tensor("b2c", [128, NF2], f32, kind="ExternalInput")
        if C8:
            b2q = nc.dram_tensor("b2q", [128, NF2], f32, kind="ExternalInput")

    Silu = mybir.ActivationFunctionType.Silu
    Ident = mybir.ActivationFunctionType.Identity

    with tile.TileContext(nc) as tc:
        with (
            tc.tile_pool(name="wpool", bufs=1) as wpool,
            tc.tile_pool(name="apool", bufs=2) as apool,
            tc.tile_pool(name="tpool", bufs=3) as tpool,
            tc.tile_pool(name="opool", bufs=4) as opool,
            tc.tile_pool(name="psA", bufs=3, space="PSUM") as psA,
            tc.tile_pool(name="psB", bufs=2, space="PSUM") as psB,
        ):
            # Resident weights / activations
            w1_sb = wpool.tile([128, KT_F, KT_D, 256], f16)
            w2_sb = wpool.tile([128, 4, KT_F, 256], f16)
            xg_sb = [wpool.tile([128, KT_D, sz], f16, name=f"xg_sb{t}")
                     for t, (_, sz) in enumerate(tiles)]
            if C8:
                w1q_sb = wpool.tile([128, KT_F, KT_D // 2, 2, 256], f8)
                w2q_sb = wpool.tile([128, 4, KT_F // 2, 2, 256], f8)
                xq8_sb = wpool.tile([128, KT_D // 2, 2, C8], f8)
                a8_sb = wpool.tile([128, KT_F // 2, 2, C8], f8)
            if not zero_b1:
                b1_sb = wpool.tile([128, NF1], f32)
            if not zero_b2:
                b2_sb = wpool.tile([128, NF2], f32)
                if C8:
                    b2q_sb = wpool.tile([128, NF2], f32)

            # Warm-up matmuls on a zeroed tile keep the PE busy (and its
            # clock ramping to 2.4GHz) during the initial DMA wait; an
            # idle PE hole >~5us here can throttle the clock to ~2GHz
            # for the whole kernel, so the count is sized to end just as
            # the first real inputs land.
            warm_sb = wpool.tile([128, 128], f16)
            nc.vector.memset(warm_sb, 0.0)
            # Dummy Silu forces the lazy ACT_TABLE_LOAD (1.3us) to happen
            # now, while scalar is idle, not at tile 0's first real silu.
            warm_s = wpool.tile([128, 1], f32)
            nc.scalar.activation(warm_s, warm_sb[:, 0:1],
                                 mybir.ActivationFunctionType.Silu)
            warm_ps = psB.tile([128, NT], f32, tag="psb")
            for _ in range(95):
                nc.tensor.matmul(warm_ps[:, :128], warm_sb, warm_sb,
                                 start=True, stop=True)

            # ---- DMA issue plan.  NOTE: TileContext schedules by data
            # deps, not python emission order — ordering within a queue
            # comes only from its FIFO, so late-needed streams (w2, fp8
            # weights) are placed BEHIND the critical stream on the same
            # queue.  Strictly two active queues at all times (a third
            # streaming queue caps the PE clock at ~2GHz).  w1 chunks
            # alternate between both queues (mm1-tile0 consumes a chunk
            # every ~2.6us; one queue delivers only ~3.5us/chunk); the
            # two k-halves of xg tile 0 head each queue.
            w1r = w1p.rearrange("i p k c -> p i k c")
            w2r = w2p.rearrange("g p k c -> p g k c")
            nc.gpsimd.dma_start(out=xg_sb[0], in_=xg_d[0][:, :, :])
            nc.sync.dma_start(out=w1_sb[:, 0, :, :], in_=w1r[:, 0, :, :])
            # xg tile 1 slots in after c9: early enough for tile 1's
            # mm1 start, late enough that every w1-chunk deadline holds.
            for i in range(1, 11, 2):
                nc.sync.dma_start(out=w1_sb[:, i, :, :], in_=w1r[:, i, :, :])
            for t in range(1, ntile):
                nc.sync.dma_start(out=xg_sb[t], in_=xg_d[t][:, :, :])
            for i in range(11, KT_F, 2):
                nc.sync.dma_start(out=w1_sb[:, i, :, :], in_=w1r[:, i, :, :])
            for i in range(2, KT_F, 2):
                nc.gpsimd.dma_start(out=w1_sb[:, i, :, :], in_=w1r[:, i, :, :])
            if not zero_b1:
                nc.sync.dma_start(out=b1_sb, in_=b1c[:, :])
            if not zero_b2:
                nc.sync.dma_start(out=b2_sb, in_=b2c[:, :])
                if C8:
                    nc.sync.dma_start(out=b2q_sb, in_=b2q[:, :])
            # w2 split across both queue tails; xq8 and the fp8 weights
            # ride the very back (needed only for the final fp8 pass) so
            # they never delay w2, which mm2-tile0 is waiting on.
            nc.gpsimd.dma_start(out=w2_sb[:, 0, :, :], in_=w2r[:, 0, :, :])
            nc.sync.dma_start(out=w2_sb[:, 2, :, :], in_=w2r[:, 2, :, :])
            nc.gpsimd.dma_start(out=w2_sb[:, 1, :, :], in_=w2r[:, 1, :, :])
            nc.sync.dma_start(out=w2_sb[:, 3, :, :], in_=w2r[:, 3, :, :])
            if C8:
                nc.sync.dma_start(out=xq8_sb, in_=xq8[:, :, :, :])
            if C8:
                w1qr = w1q.rearrange("i p k two c -> p i k two c")
                nc.gpsimd.dma_start(out=w1q_sb[:, 0:8], in_=w1qr[:, 0:8])
                nc.gpsimd.dma_start(out=w1q_sb[:, 8:16], in_=w1qr[:, 8:16])
                w2qr = w2q.rearrange("g p k two c -> p g k two c")
                nc.gpsimd.dma_start(out=w2q_sb, in_=w2qr[:, :, :, :, :])

            # ---- fp16 pass
            for t, (n0, nsz) in enumerate(tiles):
                a_t = apool.tile([128, KT_F, NTA], f16, tag="a")
                for i in range(KT_F):
                    ps1 = psA.tile([128, NT], f32, tag="ps1")
                    ps2 = psA.tile([128, NT], f32, tag="ps2")
                    for k in range(KT_D):
                        nc.tensor.matmul(
                            ps1[:, :nsz], w1_sb[:, i, k, 0:128],
                            xg_sb[t][:, k, :nsz],
                            start=(k == 0), stop=(k == KT_D - 1),
                        )
                    for k in range(KT_D):
                        nc.tensor.matmul(
                            ps2[:, :nsz], w1_sb[:, i, k, 128:256],
                            xg_sb[t][:, k, :nsz],
                            start=(k == 0), stop=(k == KT_D - 1),
                        )
                    s_t = tpool.tile([128, NTW], f32, tag="s")
                    if zero_b1:
                        nc.scalar.activation(s_t[:, :nsz], ps2[:, :nsz], Silu)
                        nc.vector.tensor_mul(
                            a_t[:, i, :nsz], ps1[:, :nsz], s_t[:, :nsz])
                    else:
                        nc.scalar.activation(
                            s_t[:, :nsz], ps2[:, :nsz], Silu,
                            bias=b1_sb[:, KT_F + i:KT_F + i + 1])
                        nc.vector.scalar_tensor_tensor(
                            a_t[:, i, :nsz], ps1[:, :nsz],
                            b1_sb[:, i:i + 1], s_t[:, :nsz],
                            mybir.AluOpType.add, Mult)
                for j in range(NF2):
                    ps = psB.tile([128, NT], f32, tag="psb")
                    for kf in range(KT_F):
                        nc.tensor.matmul(
                            ps[:, :nsz],
                            w2_sb[:, j // 2, kf, (j % 2) * 128:(j % 2) * 128 + 128],
                            a_t[:, kf, :nsz],
                            start=(kf == 0), stop=(kf == KT_F - 1),
                        )
                    o_t = opool.tile([128, NTW], f16, tag="o")
                    if zero_b2:
                        nc.scalar.activation(o_t[:, :nsz], ps[:, :nsz], Ident)
                    else:
                        nc.scalar.activation(o_t[:, :nsz], ps[:, :nsz], Ident,
                                             bias=b2_sb[:, j:j + 1])
                    # all fp16 outs ride sync: it is idle once the loads
                    # finish, while gpsimd still streams the fp8 weights
                    # — an out-DMA queued behind that bulk would delay
                    # o_t recycling and stall mm2 via the ident chain.
                    nc.sync.dma_start(out=yt_d[t][j, :, :], in_=o_t[:, :nsz])

            # ---- fp8 spill pass (DoubleRow e4m3, 2x rate)
            if C8:
                for i in range(KT_F):
                    ps1 = psA.tile([128, NT], f32, tag="ps1")
                    ps2 = psA.tile([128, NT], f32, tag="ps2")
                    for k2 in range(KT_D // 2):
                        nc.tensor.matmul(
                            ps1[:, :C8], w1q_sb[:, i, k2, :, 0:128],
                            xq8_sb[:, k2, :, :],
                            start=(k2 == 0), stop=(k2 == KT_D // 2 - 1),
                            perf_mode=DR,
                        )
                    for k2 in range(KT_D // 2):
                        nc.tensor.matmul(
                            ps2[:, :C8], w1q_sb[:, i, k2, :, 128:256],
                            xq8_sb[:, k2, :, :],
                            start=(k2 == 0), stop=(k2 == KT_D // 2 - 1),
                            perf_mode=DR,
                        )
                    s_t = tpool.tile([128, NTW], f32, tag="s")
                    if zero_b1:
                        nc.scalar.activation(s_t[:, :C8], ps2[:, :C8], Silu,
                                             scale=float(1.0 / (XS * WS)))
                    else:
                        nc.scalar.activation(s_t[:, :C8], ps2[:, :C8], Silu,
                                             scale=float(1.0 / (XS * WS)),
                                             bias=b1_sb[:, KT_F + i:KT_F + i + 1])
                        # NOTE: nonzero b1 with fp8 needs b1 un-scaled here;
                        # bias applies after scale, so this is h2 + b1b. OK.
                    # a8 = (ps1 * AS/(XS*WS)) * silu  -> e4m3
                    if zero_b1:
                        nc.vector.scalar_tensor_tensor(
                            a8_sb[:, i // 2, i % 2, :], ps1[:, :C8],
                            float(AS / (XS * WS)), s_t[:, :C8], Mult, Mult)
                    else:
                        h_t = tpool.tile([128, NTW], f32, tag="h8")
                        nc.scalar.activation(h_t[:, :C8], ps1[:, :C8], Ident,
                                             scale=float(1.0 / (XS * WS)),
                                             bias=b1_sb[:, i:i + 1])
                        nc.vector.scalar_tensor_tensor(
                            a8_sb[:, i // 2, i % 2, :], h_t[:, :C8],
                            float(AS), s_t[:, :C8], Mult, Mult)
                o8_sb = wpool.tile([128, NF2, C8], f16)
                yq8r = yq8_d.rearrange("j p c -> p j c")
                for j in range(NF2):
                    # psA (3 bufs, idle after fp8-mm1) instead of psB:
                    # with 2 bufs the fast DR j-chunks (~1us) outrun the
                    # ident chain and stall on PSUM recycling.
                    ps = psA.tile([128, NT], f32, tag="ps1")
                    for kf2 in range(KT_F // 2):
                        nc.tensor.matmul(
                            ps[:, :C8],
                            w2q_sb[:, j // 2, kf2, :,
                                   (j % 2) * 128:(j % 2) * 128 + 128],
                            a8_sb[:, kf2, :, :],
                            start=(kf2 == 0), stop=(kf2 == KT_F // 2 - 1),
                            perf_mode=DR,
                        )
                    if zero_b2:
                        nc.scalar.activation(o8_sb[:, j, :], ps[:, :C8], Ident)
                    else:
                        # bias is b2*OS (host pre-scales b2q by OS)
                        nc.scalar.activation(o8_sb[:, j, :], ps[:, :C8], Ident,
                                             bias=b2q_sb[:, j:j + 1])
                    # two batched out DMAs compress the end-of-kernel
                    # drain (each issue + queue-drain costs ~0.6us)
                    # both batches on sync: its queue stays warm from the
                    # fp16 outs, so the final descriptors prefetch instead
                    # of paying the ~2.8us cold-fetch latency; gpsimd's
                    # end-of-kernel drain is then empty.
                    if j == 5:
                        nc.sync.dma_start(out=yq8r[:, 0:6, :],
                                          in_=o8_sb[:, 0:6, :])
                    elif j == 7:
                        nc.sync.dma_start(out=yq8r[:, 6:8, :],
                                          in_=o8_sb[:, 6:8, :])
    nc.compile()
    return nc


def _get_nc(C16, C8, zero_b1, zero_b2):
    key = (C16, C8, zero_b1, zero_b2)
    nc = _NC_CACHE.get(key)
    if nc is None:
        nc = _build(C16, C8, zero_b1, zero_b2)
        _NC_CACHE[key] = nc
    return nc


def _q8(v, s):
    return np.clip(v * s, -240.0, 240.0).astype(ml_dtypes.float8_e4m3)


def _pack_weights(W1, W2):
    key = (W1.shape, W2.shape, W1.dtype.str,
           bytes(np.asarray(W1[0, 0, :8]).data),
           bytes(np.asarray(W2[0, 0, :8]).data))
    hit = _W_CACHE.get("w")
    if hit is not None and hit[0] == key:
        return hit[1]
    packs = []
    for e in range(E):
        W1T = np.ascontiguousarray(W1[e].T)            # [D, 2F] f32
        W2T = np.ascontiguousarray(W2[e].T)            # [F, D]  f32
        W1T16 = W1T.astype(np.float16)
        W2T16 = W2T.astype(np.float16)
        # w1p[i, p, k, c]: chunk i = (h1[i*128:...], h2[i*128:...])
        w1p = np.ascontiguousarray(
            W1T16.reshape(KT_D, 128, 2, KT_F, 128)
            .transpose(3, 1, 0, 2, 4).reshape(KT_F, 128, KT_D, 256))
        # w2p[g, p, k, c]: d-block g (covers output chunks 2g, 2g+1)
        w2p = np.ascontiguousarray(
            W2T16.reshape(KT_F, 128, 4, 256).transpose(2, 1, 0, 3))
        W1q = _q8(W1T, WS)
        W2q = _q8(W2T, WS)
        w1q = np.ascontiguousarray(
            W1q.reshape(KT_D // 2, 2, 128, 2, KT_F, 128)
            .transpose(4, 2, 0, 1, 3, 5).reshape(KT_F, 128, KT_D // 2, 2, 256))
        w2q = np.ascontiguousarray(
            W2q.reshape(KT_F // 2, 2, 128, 4, 256).transpose(3, 2, 0, 1, 4))
        packs.append((w1p, w2p, w1q, w2q))
    _W_CACHE["w"] = (key, packs)
    return packs


def kernel(x, Wr, temp, W1, b1, W2, b2):
    x = np.asarray(x)
    xf = np.ascontiguousarray(x.reshape(T, D), dtype=np.float32)

    # ---- host router (f64 for a stable top-k ordering)
    logits = xf.astype(np.float64) @ np.asarray(Wr).astype(np.float64).T
    logits /= np.float64(np.asarray(temp).reshape(-1)[0])
    top_idx = np.argsort(-logits, axis=1, kind="stable")[:, :K]  # [T, K]
    top_v = np.take_along_axis(logits, top_idx, axis=1)
    top_v -= top_v.max(axis=1, keepdims=True)
    exv = np.exp(top_v)
    gates = (exv / exv.sum(axis=1, keepdims=True)).astype(np.float64)  # [T, K]

    # ---- dispatch: per-expert fp16 lists (exactly C16 tokens) and fp8
    # spill lists (smallest-gate secondary pairs)
    prim_rows = [np.where(top_idx[:, 0] == e)[0] for e in range(E)]
    sec_rows = [np.where(top_idx[:, 1] == e)[0] for e in range(E)]
    counts = np.array([len(prim_rows[e]) + len(sec_rows[e]) for e in range(E)])
    maxprim = max(len(p) for p in prim_rows)
    S_tot = float((gates ** 2).sum())
    spill = SPILL
    while True:
        C16 = int(-(-max(16, counts.max() - spill) // 16) * 16)
        C16 = max(C16, int(-(-maxprim // 16) * 16), 256)
        n8s = [max(0, int(c) - C16) for c in counts]
        C8 = int(-(-max(n8s) // 16) * 16) if max(n8s) > 0 else 0
        if spill <= 0 or C8 == 0:
            break
        # predicted global rel-err from spilling the smallest-gate
        # secondary pairs (error of a pair scales with its gate)
        S8 = 0.0
        for e in range(E):
            gs = np.sort(gates[sec_rows[e], 1])[:n8s[e]]
            S8 += float((gs ** 2).sum())
        if EPS8 * np.sqrt(S8 / S_tot) <= ERR_CAP:
            break
        spill -= 32

    idx16, gate16, idx8, gate8 = [], [], [], []
    for e in range(E):
        gp = gates[prim_rows[e], 0]
        gs = gates[sec_rows[e], 1]
        order = np.argsort(gs, kind="stable")
        n8 = n8s[e]
        idx8.append(sec_rows[e][order[:n8]])
        gate8.append(gs[order[:n8]])
        idx16.append(np.concatenate([prim_rows[e], sec_rows[e][order[n8:]]]))
        gate16.append(np.concatenate([gp, gs[order[n8:]]]))

    b1a = np.asarray(b1, dtype=np.float32)
    b2a = np.asarray(b2, dtype=np.float32)
    zero_b1 = not b1a.any()
    zero_b2 = not b2a.any()
    nc = _get_nc(C16, C8, zero_b1, zero_b2)
    tiles = _tiles16(C16)

    xf16 = xf.astype(np.float16)
    packs = _pack_weights(np.asarray(W1, dtype=np.float32),
                          np.asarray(W2, dtype=np.float32))

    in_maps = []
    for e in range(E):
        w1p, w2p, w1q, w2q = packs[e]
        m = {"w1p": w1p, "w2p": w2p}
        xg = np.zeros((C16, D), np.float16)
        xg[:len(idx16[e])] = xf16[idx16[e]]
        for t, (n0, nsz) in enumerate(tiles):
            m[f"xg{t}"] = np.ascontiguousarray(
                xg[n0:n0 + nsz].reshape(nsz, KT_D, 128).transpose(2, 1, 0))
        if C8:
            x8 = np.zeros((C8, D), ml_dtypes.float8_e4m3)
            x8[:len(idx8[e])] = _q8(xf[idx8[e]], XS)
            m["xq8"] = np.ascontiguousarray(
                x8.reshape(C8, KT_D // 2, 2, 128).transpose(3, 1, 2, 0))
            m["w1q"] = w1q
            m["w2q"] = w2q
        if not zero_b1:
            m["b1c"] = np.ascontiguousarray(b1a[e].reshape(NF1, 128).T)
        if not zero_b2:
            m["b2c"] = np.ascontiguousarray(b2a[e].reshape(NF2, 128).T)
            if C8:
                m["b2q"] = np.ascontiguousarray(
                    b2a[e].reshape(NF2, 128).T * OS)
        in_maps.append(m)

    kwargs = {}
    if os.environ.get("KERNEL_TRACE"):
        kwargs = {"trace": True}
    try:
        res = run_bass_kernel_spmd(nc, in_maps, core_ids=list(range(E)), **kwargs)
    except ModuleNotFoundError:
        os.environ["BASS_NEVER_TRACE"] = "1"
        res = run_bass_kernel_spmd(nc, in_maps, core_ids=list(range(E)))
    global LAST_RESULT
    LAST_RESULT = res

    out = np.zeros((T, D), np.float64)
    for e in range(E):
        r = res.results[e]
        n16 = len(idx16[e])
        y16 = np.concatenate(
            [r[f"yt{t}"].transpose(2, 0, 1).reshape(nsz, D)
             for t, (n0, nsz) in enumerate(tiles)], axis=0)[:n16]
        out[idx16[e]] += gate16[e][:, None] * y16.astype(np.float64)
        n8 = len(idx8[e])
        if n8:
            y8 = r["yq8"].transpose(2, 0, 1).reshape(C8, D)[:n8]
            out[idx8[e]] += gate8[e][:, None] * (y8.astype(np.float64) / OS)
    return out.reshape(B, S, D).astype(np.float32)


LAST_RESULT = None


# revision 72
# speedup vs baseline: 1.0176x; 1.0021x over previous
"""MoE (top-2 of 8 experts, SwiGLU FFN) on 8 Trainium2 NeuronCores.

Strategy (expert-parallel + accuracy-guided precision split):
 - Host: router matmul (f64) + top-2 + softmax gates.  One expert per
   core.  Each expert keeps exactly C16 tokens on the fp16 path; the
   excess tokens (those with the SMALLEST gate weights, which bound
   their error contribution) spill to an fp8 (e4m3, DoubleRow 2x)
   path of capacity C8.  This simultaneously load-balances the cores
   (no fp16 padding at all) and keeps the global rel-err ~1.2e-2
   (tolerance 2e-2): error from an fp8 pair scales with its gate g,
   and the spill picks per-expert the smallest-g secondary pairs.
 - Device (per core), feature-major layout so per-feature biases are
   per-partition scalars:
     fp16 pass over C16 tokens:  hT = W1[e] @ xgT  (PE fp16->f32 PSUM)
         aT = h1 * silu(h2)   (ACT + DVE)
         yT = W2[e] @ aT      (PE, ACT -> f16 out)
     fp8 pass over C8 tokens: same, matmuls in e4m3 DoubleRow mode
         (2 contraction subtiles per pass = 2x rate), scales folded
         into the host-side quantization (x*16, W*512, a*8).
 - Host: gather back, gate-weight, scatter-add (f32).

DMA: host pre-packs every stream so each descriptor moves 2-8KB
contiguous runs per partition.  Exactly TWO HW queues (sync, gpsimd)
stream at any time — a third concurrent queue caps the PE clock at
~2GHz — with late-needed streams (w2, fp8 weights) sequenced behind
the critical ones via each queue's FIFO.  The scalar engine stays
DMA-free: queued DMA issues hit semaphore-reuse waits that would
block its activations (and with them PSUM recycling and the PE).

Shapes hardcoded for the problem: x [2,2048,1024], E=8, K=2, D=1024,
F=2048.
"""

import os

import numpy as np
import ml_dtypes

import concourse.bacc as bacc
import concourse.tile as tile
from concourse import mybir
from concourse.bass_utils import run_bass_kernel_spmd

B, S, D = 2, 2048, 1024
T = B * S
E = 8
K = 2
F = 2048
TWOF = 2 * F
KT_D = D // 128    # 8 contraction subtiles for matmul 1
KT_F = F // 128    # 16 contraction subtiles for matmul 2
NF1 = TWOF // 128  # 32 output feature chunks of matmul 1 (16 pairs)
NF2 = D // 128     # 8 output feature chunks of matmul 2
NT = 512           # fp16 token tile (matmul moving free dim)
SPILL = 288        # max fp8 spill per expert (sets C16); the host
                   # backs this off if the predicted spill error for
                   # the actual gate distribution exceeds ERR_CAP
EPS8 = 0.059       # measured rel-err of a fully-fp8 token pair
ERR_CAP = 0.0188   # error budget (tolerance 2e-2 with margin)

XS = 16.0          # x fp8 scale
WS = 512.0         # weight fp8 scale
AS = 8.0           # activation fp8 scale;  a8 = ps1*(AS/(XS*WS)) * silu
OS = AS * WS       # fp8 output scale: o8 = OS * y

_NC_CACHE = {}
_W_CACHE = {}


def _tiles16(C):
    """fp16 token tiles: a small head tile (its xg lands early so the
    PE starts sooner), remainder in near-equal <=512 chunks (multiples
    of 16).  The head tile must stay >=~350 so mm1's w1-chunk
    consumption period does not outrun the two-queue DMA delivery."""
    first = min(352, C)
    szs = [first]
    rem = C - first
    if rem > 0:
        n = -(-rem // NT)
        base = -(-(rem // n) // 16) * 16
        while rem > 0:
            sz = min(base, rem)
            if 0 < rem - sz < 64:
                sz = rem
            szs.append(sz)
            rem -= sz
    tiles = []
    off = 0
    for sz in szs:
        tiles.append((off, sz))
        off += sz
    return tiles


def _build(C16, C8, zero_b1=True, zero_b2=True):
    nc = bacc.Bacc(None, target_bir_lowering=False)
    f16, f32, f8 = mybir.dt.float16, mybir.dt.float32, mybir.dt.float8e4
    DR = mybir.MatmulPerfMode.DoubleRow
    Mult = mybir.AluOpType.mult

    tiles = _tiles16(C16)
    ntile = len(tiles)
    NTA = max(sz for _, sz in tiles)  # widest fp16 tile (SBUF sizing)
    NTW = max(NTA, C8)                # work-tile width for s/o pools

    # ---- DRAM tensors (host-packed for contiguous DMA runs)
    xg_d = [nc.dram_tensor(f"xg{t}", [128, KT_D, sz], f16, kind="ExternalInput")
            for t, (_, sz) in enumerate(tiles)]
    w1p = nc.dram_tensor("w1p", [KT_F, 128, KT_D, 256], f16, kind="ExternalInput")
    w2p = nc.dram_tensor("w2p", [4, 128, KT_F, 256], f16, kind="ExternalInput")
    yt_d = [nc.dram_tensor(f"yt{t}", [NF2, 128, sz], f16, kind="ExternalOutput")
            for t, (_, sz) in enumerate(tiles)]
    if C8:
        xq8 = nc.dram_tensor("xq8", [128, KT_D // 2, 2, C8], f8, kind="ExternalInput")
        w1q = nc.dram_tensor("w1q", [KT_F, 128, KT_D // 2, 2, 256], f8,
                             kind="ExternalInput")
        w2q = nc.dram_tensor("w2q", [4, 128, KT_F // 2, 2, 256], f8,
                             kind="ExternalInput")
        yq8_d = nc.dram_tensor("yq8", [NF2, 128, C8], f16, kind="ExternalOutput")
    if not zero_b1:
        b1c = nc.dram_tensor("b1c", [128, NF1], f32, kind="ExternalInput")
    if not zero_b2:
        b2c = nc.dram_tensor("b2c", [128, NF2], f32, kind="ExternalInput")
        if C8:
            b2q = nc.dram_tensor("b2q", [128, NF2], f32, kind="ExternalInput")

    Silu = mybir.ActivationFunctionType.Silu
    Ident = mybir.ActivationFunctionType.Identity

    with tile.TileContext(nc) as tc:
        with (
            tc.tile_pool(name="wpool", bufs=1) as wpool,
            tc.tile_pool(name="apool", bufs=2) as apool,
            tc.tile_pool(name="tpool", bufs=3) as tpool,
            tc.tile_pool(name="opool", bufs=4) as opool,
            tc.tile_pool(name="psA", bufs=3, space="PSUM") as psA,
            tc.tile_pool(name="psB", bufs=2, space="PSUM") as psB,
        ):
            # Resident weights / activations
            w1_sb = wpool.tile([128, KT_F, KT_D, 256], f16)
            w2_sb = wpool.tile([128, 4, KT_F, 256], f16)
            xg_sb = [wpool.tile([128, KT_D, sz], f16, name=f"xg_sb{t}")
                     for t, (_, sz) in enumerate(tiles)]
            if C8:
                w1q_sb = wpool.tile([128, KT_F, KT_D // 2, 2, 256], f8)
                w2q_sb = wpool.tile([128, 4, KT_F // 2, 2, 256], f8)
                xq8_sb = wpool.tile([128, KT_D // 2, 2, C8], f8)
                a8_sb = wpool.tile([128, KT_F // 2, 2, C8], f8)
            if not zero_b1:
                b1_sb = wpool.tile([128, NF1], f32)
            if not zero_b2:
                b2_sb = wpool.tile([128, NF2], f32)
                if C8:
                    b2q_sb = wpool.tile([128, NF2], f32)

            # Warm-up matmuls on a zeroed tile keep the PE busy (and its
            # clock ramping to 2.4GHz) during the initial DMA wait; an
            # idle PE hole >~5us here can throttle the clock to ~2GHz
            # for the whole kernel, so the count is sized to end just as
            # the first real inputs land.
            warm_sb = wpool.tile([128, 128], f16)
            nc.vector.memset(warm_sb, 0.0)
            # Dummy Silu forces the lazy ACT_TABLE_LOAD (1.3us) to happen
            # now, while scalar is idle, not at tile 0's first real silu.
            warm_s = wpool.tile([128, 1], f32)
            nc.scalar.activation(warm_s, warm_sb[:, 0:1],
                                 mybir.ActivationFunctionType.Silu)
            warm_ps = psB.tile([128, NT], f32, tag="psb")
            for _ in range(95):
                nc.tensor.matmul(warm_ps[:, :128], warm_sb, warm_sb,
                                 start=True, stop=True)

            # ---- DMA issue plan.  NOTE: TileContext schedules by data
            # deps, not python emission order — ordering within a queue
            # comes only from its FIFO, so late-needed streams (w2, fp8
            # weights) are placed BEHIND the critical stream on the same
            # queue.  Strictly two active queues at all times (a third
            # streaming queue caps the PE clock at ~2GHz).  w1 chunks
            # alternate between both queues (mm1-tile0 consumes a chunk
            # every ~2.6us; one queue delivers only ~3.5us/chunk); the
            # two k-halves of xg tile 0 head each queue.
            w1r = w1p.rearrange("i p k c -> p i k c")
            w2r = w2p.rearrange("g p k c -> p g k c")
            nc.gpsimd.dma_start(out=xg_sb[0], in_=xg_d[0][:, :, :])
            nc.sync.dma_start(out=w1_sb[:, 0, :, :], in_=w1r[:, 0, :, :])
            # xg tile 1 slots in after c9: early enough for tile 1's
            # mm1 start, late enough that every w1-chunk deadline holds.
            for i in range(1, 11, 2):
                nc.sync.dma_start(out=w1_sb[:, i, :, :], in_=w1r[:, i, :, :])
            for t in range(1, ntile):
                nc.sync.dma_start(out=xg_sb[t], in_=xg_d[t][:, :, :])
            for i in range(11, KT_F, 2):
                nc.sync.dma_start(out=w1_sb[:, i, :, :], in_=w1r[:, i, :, :])
            for i in range(2, KT_F, 2):
                nc.gpsimd.dma_start(out=w1_sb[:, i, :, :], in_=w1r[:, i, :, :])
            if not zero_b1:
                nc.sync.dma_start(out=b1_sb, in_=b1c[:, :])
            if not zero_b2:
                nc.sync.dma_start(out=b2_sb, in_=b2c[:, :])
                if C8:
                    nc.sync.dma_start(out=b2q_sb, in_=b2q[:, :])
            # w2 split across both queue tails; xq8 and the fp8 weights
            # ride the very back (needed only for the final fp8 pass) so
            # they never delay w2, which mm2-tile0 is waiting on.
            nc.gpsimd.dma_start(out=w2_sb[:, 0, :, :], in_=w2r[:, 0, :, :])
            nc.sync.dma_start(out=w2_sb[:, 2, :, :], in_=w2r[:, 2, :, :])
            nc.gpsimd.dma_start(out=w2_sb[:, 1, :, :], in_=w2r[:, 1, :, :])
            nc.sync.dma_start(out=w2_sb[:, 3, :, :], in_=w2r[:, 3, :, :])
            if C8:
                nc.sync.dma_start(out=xq8_sb, in_=xq8[:, :, :, :])
            if C8:
                w1qr = w1q.rearrange("i p k two c -> p i k two c")
                nc.gpsimd.dma_start(out=w1q_sb[:, 0:8], in_=w1qr[:, 0:8])
                nc.gpsimd.dma_start(out=w1q_sb[:, 8:16], in_=w1qr[:, 8:16])
                w2qr = w2q.rearrange("g p k two c -> p g k two c")
                nc.gpsimd.dma_start(out=w2q_sb, in_=w2qr[:, :, :, :, :])

            # ---- fp16 pass
            for t, (n0, nsz) in enumerate(tiles):
                a_t = apool.tile([128, KT_F, NTA], f16, tag="a")
                for i in range(KT_F):
                    ps1 = psA.tile([128, NT], f32, tag="ps1")
                    ps2 = psA.tile([128, NT], f32, tag="ps2")
                    for k in range(KT_D):
                        nc.tensor.matmul(
                            ps1[:, :nsz], w1_sb[:, i, k, 0:128],
                            xg_sb[t][:, k, :nsz],
                            start=(k == 0), stop=(k == KT_D - 1),
                        )
                    for k in range(KT_D):
                        nc.tensor.matmul(
                            ps2[:, :nsz], w1_sb[:, i, k, 128:256],
                            xg_sb[t][:, k, :nsz],
                            start=(k == 0), stop=(k == KT_D - 1),
                        )
                    s_t = tpool.tile([128, NTW], f32, tag="s")
                    if zero_b1:
                        nc.scalar.activation(s_t[:, :nsz], ps2[:, :nsz], Silu)
                        nc.vector.tensor_mul(
                            a_t[:, i, :nsz], ps1[:, :nsz], s_t[:, :nsz])
                    else:
                        nc.scalar.activation(
                            s_t[:, :nsz], ps2[:, :nsz], Silu,
                            bias=b1_sb[:, KT_F + i:KT_F + i + 1])
                        nc.vector.scalar_tensor_tensor(
                            a_t[:, i, :nsz], ps1[:, :nsz],
                            b1_sb[:, i:i + 1], s_t[:, :nsz],
                            mybir.AluOpType.add, Mult)
                for j in range(NF2):
                    ps = psB.tile([128, NT], f32, tag="psb")
                    for kf in range(KT_F):
                        nc.tensor.matmul(
                            ps[:, :nsz],
                            w2_sb[:, j // 2, kf, (j % 2) * 128:(j % 2) * 128 + 128],
                            a_t[:, kf, :nsz],
                            start=(kf == 0), stop=(kf == KT_F - 1),
                        )
                    o_t = opool.tile([128, NTW], f16, tag="o")
                    if zero_b2:
                        nc.scalar.activation(o_t[:, :nsz], ps[:, :nsz], Ident)
                    else:
                        nc.scalar.activation(o_t[:, :nsz], ps[:, :nsz], Ident,
                                             bias=b2_sb[:, j:j + 1])
                    # all fp16 outs ride sync: it is idle once the loads
                    # finish, while gpsimd still streams the fp8 weights
                    # — an out-DMA queued behind that bulk would delay
                    # o_t recycling and stall mm2 via the ident chain.
                    nc.sync.dma_start(out=yt_d[t][j, :, :], in_=o_t[:, :nsz])

            # ---- fp8 spill pass (DoubleRow e4m3, 2x rate)
            if C8:
                for i in range(KT_F):
                    ps1 = psA.tile([128, NT], f32, tag="ps1")
                    ps2 = psA.tile([128, NT], f32, tag="ps2")
                    for k2 in range(KT_D // 2):
                        nc.tensor.matmul(
                            ps1[:, :C8], w1q_sb[:, i, k2, :, 0:128],
                            xq8_sb[:, k2, :, :],
                            start=(k2 == 0), stop=(k2 == KT_D // 2 - 1),
                            perf_mode=DR,
                        )
                    for k2 in range(KT_D // 2):
                        nc.tensor.matmul(
                            ps2[:, :C8], w1q_sb[:, i, k2, :, 128:256],
                            xq8_sb[:, k2, :, :],
                            start=(k2 == 0), stop=(k2 == KT_D // 2 - 1),
                            perf_mode=DR,
                        )
                    s_t = tpool.tile([128, NTW], f32, tag="s")
                    if zero_b1:
                        nc.scalar.activation(s_t[:, :C8], ps2[:, :C8], Silu,
                                             scale=float(1.0 / (XS * WS)))
                    else:
                        nc.scalar.activation(s_t[:, :C8], ps2[:, :C8], Silu,
                                             scale=float(1.0 / (XS * WS)),
                                             bias=b1_sb[:, KT_F + i:KT_F + i + 1])
                        # NOTE: nonzero b1 with fp8 needs b1 un-scaled here;
                        # bias applies after scale, so this is h2 + b1b. OK.
                    # a8 = (ps1 * AS/(XS*WS)) * silu  -> e4m3
                    if zero_b1:
                        nc.vector.scalar_tensor_tensor(
                            a8_sb[:, i // 2, i % 2, :], ps1[:, :C8],
                            float(AS / (XS * WS)), s_t[:, :C8], Mult, Mult)
                    else:
                        h_t = tpool.tile([128, NTW], f32, tag="h8")
                        nc.scalar.activation(h_t[:, :C8], ps1[:, :C8], Ident,
                                             scale=float(1.0 / (XS * WS)),
                                             bias=b1_sb[:, i:i + 1])
                        nc.vector.scalar_tensor_tensor(
                            a8_sb[:, i // 2, i % 2, :], h_t[:, :C8],
                            float(AS), s_t[:, :C8], Mult, Mult)
                o8_sb = wpool.tile([128, NF2, C8], f16)
                yq8r = yq8_d.rearrange("j p c -> p j c")
                for j in range(NF2):
                    # psA (3 bufs, idle after fp8-mm1) instead of psB:
                    # with 2 bufs the fast DR j-chunks (~1us) outrun the
                    # ident chain and stall on PSUM recycling.
                    ps = psA.tile([128, NT], f32, tag="ps1")
                    for kf2 in range(KT_F // 2):
                        nc.tensor.matmul(
                            ps[:, :C8],
                            w2q_sb[:, j // 2, kf2, :,
                                   (j % 2) * 128:(j % 2) * 128 + 128],
                            a8_sb[:, kf2, :, :],
                            start=(kf2 == 0), stop=(kf2 == KT_F // 2 - 1),
                            perf_mode=DR,
                        )
                    if zero_b2:
                        # the last two idents ride the idle DVE so they
                        # run in parallel with scalar's chain and the
                        # final out DMA fires sooner
                        if j >= 6:
                            nc.vector.tensor_copy(o8_sb[:, j, :], ps[:, :C8])
                        else:
                            nc.scalar.activation(o8_sb[:, j, :], ps[:, :C8],
                                                 Ident)
                    else:
                        # bias is b2*OS (host pre-scales b2q by OS)
                        nc.scalar.activation(o8_sb[:, j, :], ps[:, :C8], Ident,
                                             bias=b2q_sb[:, j:j + 1])
                    # two batched out DMAs compress the end-of-kernel
                    # drain (each issue + queue-drain costs ~0.6us)
                    # both batches on sync: its queue stays warm from the
                    # fp16 outs, so the final descriptors prefetch instead
                    # of paying the ~2.8us cold-fetch latency; gpsimd's
                    # end-of-kernel drain is then empty.
                    if j == 5:
                        nc.sync.dma_start(out=yq8r[:, 0:6, :],
                                          in_=o8_sb[:, 0:6, :])
                    elif j == 7:
                        nc.sync.dma_start(out=yq8r[:, 6:8, :],
                                          in_=o8_sb[:, 6:8, :])
    nc.compile()
    return nc


def _get_nc(C16, C8, zero_b1, zero_b2):
    key = (C16, C8, zero_b1, zero_b2)
    nc = _NC_CACHE.get(key)
    if nc is None:
        nc = _build(C16, C8, zero_b1, zero_b2)
        _NC_CACHE[key] = nc
    return nc


def _q8(v, s):
    return np.clip(v * s, -240.0, 240.0).astype(ml_dtypes.float8_e4m3)


def _pack_weights(W1, W2):
    key = (W1.shape, W2.shape, W1.dtype.str,
           bytes(np.asarray(W1[0, 0, :8]).data),
           bytes(np.asarray(W2[0, 0, :8]).data))
    hit = _W_CACHE.get("w")
    if hit is not None and hit[0] == key:
        return hit[1]
    packs = []
    for e in range(E):
        W1T = np.ascontiguousarray(W1[e].T)            # [D, 2F] f32
        W2T = np.ascontiguousarray(W2[e].T)            # [F, D]  f32
        W1T16 = W1T.astype(np.float16)
        W2T16 = W2T.astype(np.float16)
        # w1p[i, p, k, c]: chunk i = (h1[i*128:...], h2[i*128:...])
        w1p = np.ascontiguousarray(
            W1T16.reshape(KT_D, 128, 2, KT_F, 128)
            .transpose(3, 1, 0, 2, 4).reshape(KT_F, 128, KT_D, 256))
        # w2p[g, p, k, c]: d-block g (covers output chunks 2g, 2g+1)
        w2p = np.ascontiguousarray(
            W2T16.reshape(KT_F, 128, 4, 256).transpose(2, 1, 0, 3))
        W1q = _q8(W1T, WS)
        W2q = _q8(W2T, WS)
        w1q = np.ascontiguousarray(
            W1q.reshape(KT_D // 2, 2, 128, 2, KT_F, 128)
            .transpose(4, 2, 0, 1, 3, 5).reshape(KT_F, 128, KT_D // 2, 2, 256))
        w2q = np.ascontiguousarray(
            W2q.reshape(KT_F // 2, 2, 128, 4, 256).transpose(3, 2, 0, 1, 4))
        packs.append((w1p, w2p, w1q, w2q))
    _W_CACHE["w"] = (key, packs)
    return packs


def kernel(x, Wr, temp, W1, b1, W2, b2):
    x = np.asarray(x)
    xf = np.ascontiguousarray(x.reshape(T, D), dtype=np.float32)

    # ---- host router (f64 for a stable top-k ordering)
    logits = xf.astype(np.float64) @ np.asarray(Wr).astype(np.float64).T
    logits /= np.float64(np.asarray(temp).reshape(-1)[0])
    top_idx = np.argsort(-logits, axis=1, kind="stable")[:, :K]  # [T, K]
    top_v = np.take_along_axis(logits, top_idx, axis=1)
    top_v -= top_v.max(axis=1, keepdims=True)
    exv = np.exp(top_v)
    gates = (exv / exv.sum(axis=1, keepdims=True)).astype(np.float64)  # [T, K]

    # ---- dispatch: per-expert fp16 lists (exactly C16 tokens) and fp8
    # spill lists (smallest-gate secondary pairs)
    prim_rows = [np.where(top_idx[:, 0] == e)[0] for e in range(E)]
    sec_rows = [np.where(top_idx[:, 1] == e)[0] for e in range(E)]
    counts = np.array([len(prim_rows[e]) + len(sec_rows[e]) for e in range(E)])
    maxprim = max(len(p) for p in prim_rows)
    S_tot = float((gates ** 2).sum())
    spill = SPILL
    while True:
        C16 = int(-(-max(16, counts.max() - spill) // 16) * 16)
        C16 = max(C16, int(-(-maxprim // 16) * 16), 256)
        n8s = [max(0, int(c) - C16) for c in counts]
        C8 = int(-(-max(n8s) // 16) * 16) if max(n8s) > 0 else 0
        if spill <= 0 or C8 == 0:
            break
        # predicted global rel-err from spilling the smallest-gate
        # secondary pairs (error of a pair scales with its gate)
        S8 = 0.0
        for e in range(E):
            gs = np.sort(gates[sec_rows[e], 1])[:n8s[e]]
            S8 += float((gs ** 2).sum())
        if EPS8 * np.sqrt(S8 / S_tot) <= ERR_CAP:
            break
        spill -= 32

    idx16, gate16, idx8, gate8 = [], [], [], []
    for e in range(E):
        gp = gates[prim_rows[e], 0]
        gs = gates[sec_rows[e], 1]
        order = np.argsort(gs, kind="stable")
        n8 = n8s[e]
        idx8.append(sec_rows[e][order[:n8]])
        gate8.append(gs[order[:n8]])
        idx16.append(np.concatenate([prim_rows[e], sec_rows[e][order[n8:]]]))
        gate16.append(np.concatenate([gp, gs[order[n8:]]]))

    b1a = np.asarray(b1, dtype=np.float32)
    b2a = np.asarray(b2, dtype=np.float32)
    zero_b1 = not b1a.any()
    zero_b2 = not b2a.any()
    nc = _get_nc(C16, C8, zero_b1, zero_b2)
    tiles = _tiles16(C16)

    xf16 = xf.astype(np.float16)
    packs = _pack_weights(np.asarray(W1, dtype=np.float32),
                          np.asarray(W2, dtype=np.float32))

    in_maps = []
    for e in range(E):
        w1p, w2p, w1q, w2q = packs[e]
        m = {"w1p": w1p, "w2p": w2p}
        xg = np.zeros((C16, D), np.float16)
        xg[:len(idx16[e])] = xf16[idx16[e]]
        for t, (n0, nsz) in enumerate(tiles):
            m[f"xg{t}"] = np.ascontiguousarray(
                xg[n0:n0 + nsz].reshape(nsz, KT_D, 128).transpose(2, 1, 0))
        if C8:
            x8 = np.zeros((C8, D), ml_dtypes.float8_e4m3)
            x8[:len(idx8[e])] = _q8(xf[idx8[e]], XS)
            m["xq8"] = np.ascontiguousarray(
                x8.reshape(C8, KT_D // 2, 2, 128).transpose(3, 1, 2, 0))
            m["w1q"] = w1q
            m["w2q"] = w2q
        if not zero_b1:
            m["b1c"] = np.ascontiguousarray(b1a[e].reshape(NF1, 128).T)
        if not zero_b2:
            m["b2c"] = np.ascontiguousarray(b2a[e].reshape(NF2, 128).T)
            if C8:
                m["b2q"] = np.ascontiguousarray(
                    b2a[e].reshape(NF2, 128).T * OS)
        in_maps.append(m)

    kwargs = {}
    if os.environ.get("KERNEL_TRACE"):
        kwargs = {"trace": True}
    try:
        res = run_bass_kernel_spmd(nc, in_maps, core_ids=list(range(E)), **kwargs)
    except ModuleNotFoundError:
        os.environ["BASS_NEVER_TRACE"] = "1"
        res = run_bass_kernel_spmd(nc, in_maps, core_ids=list(range(E)))
    global LAST_RESULT
    LAST_RESULT = res

    out = np.zeros((T, D), np.float64)
    for e in range(E):
        r = res.results[e]
        n16 = len(idx16[e])
        y16 = np.concatenate(
            [r[f"yt{t}"].transpose(2, 0, 1).reshape(nsz, D)
             for t, (n0, nsz) in enumerate(tiles)], axis=0)[:n16]
        out[idx16[e]] += gate16[e][:, None] * y16.astype(np.float64)
        n8 = len(idx8[e])
        if n8:
            y8 = r["yq8"].transpose(2, 0, 1).reshape(C8, D)[:n8]
            out[idx8[e]] += gate8[e][:, None] * (y8.astype(np.float64) / OS)
    return out.reshape(B, S, D).astype(np.float32)


LAST_RESULT = None
